# revision 35
# baseline (speedup 1.0000x reference)
"""CostVolume (81-displacement, L2-normalized, leaky-relu) Trainium2 kernel v3.

Full inputs (B=4, C=128, H=128, W=256) sharded across 8 NeuronCores:
batch x H-half data parallel (core k -> b=k//2, h-half=k%2). The +-4 halo is
handled host-side by slicing a zero-padded feat2; no collectives.

v3 design (vs v2: compact f1 norms + scale-in-evac, end-of-kernel dump):
  - all I/O in bf16; f2 shipped TRANSPOSED [C, W'=264, H'=72] so the gram
    matmul streams its rhs in (u, dy) order; 4x col-tiled 32-pixel matmuls.
  - BOTH f1 and f2 normalized on-chip symmetrically: 1024-col chunks of
    square (DVE bf16 2x / some chunks on idle GpSimd) -> ones-matmul channel
    sums (PSUM, broadcast) -> one Abs_reciprocal_sqrt ACT pass (bf16 out) ->
    one DVE bf16 multiply in place.  Software-pipelined emission (mul lags
    one chunk) so no engine FIFO head-blocks.
  - evac is pure leaky_relu: first ACT_ROWS rows of each 16-row band chunk
    via 1-pass ACT Prelu; the rest via 1-pass DVE copy (tensor_scalar 2x)
    plus a deferred 2-pass big-tile leaky (t=0.1x 4x; max(x,t) 2x) on the
    contiguous block.
  - band [128, (g h) * 360] dumped as contiguous 16-row chunk DMAs issued
    as evac completes (overlapped); last chunk in 4-row pieces to cut the
    tail. Host extracts the 9 per-pixel diagonals from the full band.
"""
import numpy as np
import ml_dtypes

import concourse.bass as bass
import concourse.bacc as bacc
import concourse.tile as tile
from concourse import mybir
from concourse.bass_utils import run_bass_kernel_spmd

F32 = mybir.dt.float32
BF16 = mybir.dt.bfloat16
BF16_NP = ml_dtypes.bfloat16

B, C, H, W = 4, 128, 128, 256
D = 4
HS = 64                    # h rows per core
HP, WP = HS + 2 * D, W + 2 * D   # padded f2 shard dims: 72, 264
N2 = WP * HP               # 19008 flat f2 elements per partition
N1 = HS * W                # 16384 flat f1 elements per partition
CH = 1024                  # norm pipeline chunk
ACT_ROWS = 10              # rows per 16-row band chunk evacuated on ACT

_CACHE = {}


def _build():
    nc = bacc.Bacc("TRN2", target_bir_lowering=False, debug=False)

    f1_d = nc.dram_tensor("f1", [C, HS, W], BF16, kind="ExternalInput")
    f2_d = nc.dram_tensor("f2t", [C, WP, HP], BF16, kind="ExternalInput")
    bo_d = nc.dram_tensor("bandout", [128, 2 * HS * 360], BF16,
                          kind="ExternalOutput")

    with tile.TileContext(nc) as tc:
        with (
            tc.tile_pool(name="pers", bufs=1) as pers,
            tc.tile_pool(name="sqp", bufs=3) as sqp,
            tc.tile_pool(name="invp", bufs=3) as invp,
            tc.tile_pool(name="evp", bufs=2) as evp,
            tc.tile_pool(name="small", bufs=1) as smallp,
            tc.tile_pool(name="psb", bufs=4, space="PSUM") as psb,
            tc.tile_pool(name="psn", bufs=2, space="PSUM") as psn,
        ):
            ones128 = smallp.tile([128, 128], BF16)
            nc.vector.memset(ones128[:], 1.0)
            eps128 = smallp.tile([128, 1], F32)
            nc.vector.memset(eps128[:], 1e-20)

            # ---- inputs: fine pieces ordered for earliest compute start --
            f1 = pers.tile([C, HS, W], BF16)
            f2t = pers.tile([C, WP, HP], BF16)

            def f2_piece(i, eng):   # 8 pieces of 33 w-cols
                eng.dma_start(out=f2t[:, 33 * i:33 * i + 33, :],
                              in_=f2_d[:, 33 * i:33 * i + 33, :])

            def f1_piece(i, eng):   # 8 pieces of 8 h-rows
                eng.dma_start(out=f1[:, 8 * i:8 * i + 8, :],
                              in_=f1_d[:, 8 * i:8 * i + 8, :])

            # one FIFO ring drains in-order at full bandwidth: issue all
            # pieces on sync, critical-path data (f2 w<165, f1 h<8) first
            f2_piece(0, nc.sync)
            f1_piece(0, nc.sync)
            for i in range(1, 5):
                f2_piece(i, nc.sync)
            f1_piece(1, nc.sync)
            for i in range(5, 8):
                f2_piece(i, nc.sync)
            for i in range(2, 8):
                f1_piece(i, nc.sync)
            f1f = f1[:].rearrange("c h w -> c (h w)")
            f2f = f2t[:].rearrange("c w h -> c (w h)")

            band = pers.tile([128, 2 * HS * 360], BF16)

            # ---- normalization chunk pipeline ----
            # chunk list: (src_view, start, n, scale, sq_on_gpsimd)
            n2c = (N2 + CH - 1) // CH          # 19
            n1c = N1 // CH                     # 16
            chunks = []
            for i in range(n2c):
                n = min(CH, N2 - CH * i)
                # alternate f2 squares onto gpsimd (queue is otherwise idle)
                chunks.append(("f2", CH * i, n, 1.0, i % 2 == 0, False))
            for i in range(n1c):
                chunks.append(("f1", CH * i, CH, float(C * C),
                               i % 3 == 2, False))
            # emission order: f2[0:10] + f1[0] first (gram critical path),
            # rest interleaved into the gram loop.
            # rest: f1 chunks FIRST — gram (0, h) needs f1 chunk h//4
            # normalized by gram-group h (f1 chunk k lands at group k-1).
            head = chunks[:10] + [chunks[n2c]]
            rest = chunks[n2c + 1:] + chunks[10:n2c]

            pending = []

            def emit_norm(ck):
                kind, s0, n, scale, on_gps, mul_gps = ck
                src = f2f if kind == "f2" else f1f
                sq = sqp.tile([128, CH], BF16)
                eng = nc.gpsimd if on_gps else nc.vector
                eng.tensor_mul(out=sq[:, 0:n], in0=src[:, s0:s0 + n],
                               in1=src[:, s0:s0 + n])
                ps = psn.tile([128, CH], F32)
                for off in range(0, n, 512):
                    m = min(512, n - off)
                    nc.tensor.matmul(ps[:, off:off + m], ones128[:],
                                     sq[:, off:off + m], start=True,
                                     stop=True)
                inv = invp.tile([128, CH], BF16)
                nc.scalar.activation(
                    out=inv[:, 0:n], in_=ps[:, 0:n],
                    func=mybir.ActivationFunctionType.Abs_reciprocal_sqrt,
                    scale=scale, bias=eps128[:])
                pending.append((src, s0, n, inv, mul_gps))

            def flush_mul():
                src, s0, n, inv, mul_gps = pending.pop(0)
                eng = nc.gpsimd if mul_gps else nc.vector
                eng.tensor_mul(out=src[:, s0:s0 + n],
                               in0=src[:, s0:s0 + n],
                               in1=inv[:, 0:n])

            for ck in head:
                emit_norm(ck)
                if len(pending) > 1:
                    flush_mul()
            while pending:      # f1 chunk 0 must be normalized pre-gram
                flush_mul()

            # ---- gram sweep + evac + chunked dump ----
            f2ap = f2t[:]

            def leaky_blk(b0, b1):
                blk = band[:, b0 * 360:b1 * 360]
                t = evp.tile([128, (b1 - b0) * 360], BF16)
                nc.vector.tensor_scalar_mul(out=t[:], in0=blk, scalar1=0.1)
                nc.vector.tensor_tensor(out=blk, in0=blk, in1=t[:],
                                        op=mybir.AluOpType.max)

            rest_i = 0
            for g in range(2):
                for h in range(HS):
                    # interleave remaining norm chunks every other group so
                    # the tensor FIFO isn't throttled by sum-matmuls
                    grp = HS * g + h
                    if rest_i < len(rest) and grp % 2 == 0:
                        emit_norm(rest[rest_i])
                        rest_i += 1
                    elif pending and grp % 2 == 1:
                        flush_mul()
                    pband = psb.tile([128, 360], F32)
                    for j in range(4):
                        w0 = 128 * g + 32 * j
                        rhs = bass.AP(
                            tensor=f2ap.tensor,
                            offset=f2ap.offset + w0 * HP + h,
                            ap=[[WP * HP, 128], [HP, 40], [1, 9]],
                        )
                        nc.tensor.matmul(
                            pband[32 * j:32 * j + 32, :],
                            f1[:, h, w0:w0 + 32],
                            rhs, start=True, stop=True,
                            tile_position=(0, 32 * j))
                    row = HS * g + h
                    r16 = h % 16
                    last_chunk = (g == 1 and h >= HS - 16)
                    if last_chunk:
                        # DVE rows first (+deferred leaky), ACT rows last,
                        # fine-grained dumps -> short tail
                        dst = band[:, row * 360:row * 360 + 360]
                        if r16 < 8:
                            nc.vector.tensor_scalar_mul(
                                out=dst, in0=pband[:], scalar1=1.0)
                        else:
                            nc.scalar.activation(
                                out=dst, in_=pband[:],
                                func=mybir.ActivationFunctionType.Prelu,
                                alpha=0.1)
                        if r16 == 7:
                            leaky_blk(112, 120)
                            nc.sync.dma_start(
                                out=bo_d[:, 112 * 360:120 * 360],
                                in_=band[:, 112 * 360:120 * 360])
                        elif r16 in (11, 15):
                            c0 = (120 if r16 == 11 else 124) * 360
                            c1 = (row + 1) * 360
                            nc.sync.dma_start(out=bo_d[:, c0:c1],
                                              in_=band[:, c0:c1])
                        continue
                    dst = band[:, row * 360:row * 360 + 360]
                    if r16 < ACT_ROWS:
                        nc.scalar.activation(
                            out=dst, in_=pband[:],
                            func=mybir.ActivationFunctionType.Prelu,
                            alpha=0.1)
                    else:
                        nc.vector.tensor_scalar_mul(
                            out=dst, in0=pband[:], scalar1=1.0)
                    if r16 == 15:
                        leaky_blk(row - (15 - ACT_ROWS), row + 1)
                        c0 = (row - 15) * 360
                        nc.sync.dma_start(
                            out=bo_d[:, c0:(row + 1) * 360],
                            in_=band[:, c0:(row + 1) * 360])
            while pending:
                flush_mul()

    nc.compile()
    return nc


def _get_nc():
    if "nc" not in _CACHE:
        _CACHE["nc"] = _build()
    return _CACHE["nc"]


def _shard_inputs(feat1, feat2_warped):
    feat1 = np.asarray(feat1, dtype=np.float32)
    feat2 = np.asarray(feat2_warped, dtype=np.float32)
    f2pad = np.pad(feat2, ((0, 0), (0, 0), (D, D), (D, D)))
    in_maps = []
    for k in range(8):
        b, s = k // 2, k % 2
        f1s = feat1[b, :, HS * s: HS * s + HS, :].astype(BF16_NP)
        f2s = f2pad[b, :, HS * s: HS * s + HP, :]          # [C, 72, 264]
        f2s = np.ascontiguousarray(
            f2s.transpose(0, 2, 1)).astype(BF16_NP)        # [C, 264, 72]
        in_maps.append({"f1": np.ascontiguousarray(f1s), "f2t": f2s})
    return in_maps


_GIDX = (9 * (np.arange(128) % 32))[:, None, None, None] + \
    np.arange(81)[None, None, None, :]  # [128,1,1,81]


def _gather(results):
    out = np.empty((B, 81, H, W), dtype=np.float32)
    for k in range(8):
        b, s = k // 2, k % 2
        band = results[k]["bandout"].astype(np.float32)  # [128, 2*HS*360]
        band = band.reshape(128, 2, HS, 360)             # p, g, h, (u' dy)
        # pixel w = 128g + p; value (dy,dx) at col (p%32 + dx)*9 + dy
        bw = np.take_along_axis(band, _GIDX, axis=3)     # [128,2,HS,81]
        bw = bw.reshape(128, 2, HS, 9, 9)                # p g h dx dy
        core = bw.transpose(4, 3, 2, 1, 0).reshape(81, HS, W)
        out[b, :, HS * s: HS * s + HS, :] = core
    return out


def run(feat1, feat2_warped, trace=False):
    nc = _get_nc()
    in_maps = _shard_inputs(feat1, feat2_warped)
    res = run_bass_kernel_spmd(nc, in_maps, list(range(8)), trace=trace)
    return _gather(res.results), res


def kernel(feat1, feat2_warped):
    out, _ = run(feat1, feat2_warped)
    return out


# revision 38
# speedup vs baseline: 1.0102x; 1.0102x over previous
"""CostVolume (81-displacement, L2-normalized, leaky-relu) Trainium2 kernel v3.

Full inputs (B=4, C=128, H=128, W=256) sharded across 8 NeuronCores:
batch x H-half data parallel (core k -> b=k//2, h-half=k%2). The +-4 halo is
handled host-side by slicing a zero-padded feat2; no collectives.

v3 design (vs v2: compact f1 norms + scale-in-evac, end-of-kernel dump):
  - all I/O in bf16; f2 shipped TRANSPOSED [C, W'=264, H'=72] so the gram
    matmul streams its rhs in (u, dy) order; 4x col-tiled 32-pixel matmuls.
  - BOTH f1 and f2 normalized on-chip symmetrically: 1024-col chunks of
    square (DVE bf16 2x / some chunks on idle GpSimd) -> ones-matmul channel
    sums (PSUM, broadcast) -> one Abs_reciprocal_sqrt ACT pass (bf16 out) ->
    one DVE bf16 multiply in place.  Software-pipelined emission (mul lags
    one chunk) so no engine FIFO head-blocks.
  - evac is pure leaky_relu: first ACT_ROWS rows of each 16-row band chunk
    via 1-pass ACT Prelu; the rest via 1-pass DVE copy (tensor_scalar 2x)
    plus a deferred 2-pass big-tile leaky (t=0.1x 4x; max(x,t) 2x) on the
    contiguous block.
  - band [128, (g h) * 360] dumped as contiguous 16-row chunk DMAs issued
    as evac completes (overlapped); last chunk in 4-row pieces to cut the
    tail. Host extracts the 9 per-pixel diagonals from the full band.
"""
import numpy as np
import ml_dtypes

import concourse.bass as bass
import concourse.bacc as bacc
import concourse.tile as tile
from concourse import mybir
from concourse.bass_utils import run_bass_kernel_spmd

F32 = mybir.dt.float32
BF16 = mybir.dt.bfloat16
BF16_NP = ml_dtypes.bfloat16

B, C, H, W = 4, 128, 128, 256
D = 4
HS = 64                    # h rows per core
HP, WP = HS + 2 * D, W + 2 * D   # padded f2 shard dims: 72, 264
N2 = WP * HP               # 19008 flat f2 elements per partition
N1 = HS * W                # 16384 flat f1 elements per partition
CH = 1024                  # norm pipeline chunk
ACT_ROWS = 10              # rows per 16-row band chunk evacuated on ACT

_CACHE = {}


def _build():
    nc = bacc.Bacc("TRN2", target_bir_lowering=False, debug=False)

    f1_d = nc.dram_tensor("f1", [C, HS, W], BF16, kind="ExternalInput")
    f2_d = nc.dram_tensor("f2t", [C, WP, HP], BF16, kind="ExternalInput")
    bo_d = nc.dram_tensor("bandout", [128, 2 * HS * 360], BF16,
                          kind="ExternalOutput")

    with tile.TileContext(nc) as tc:
        with (
            tc.tile_pool(name="pers", bufs=1) as pers,
            tc.tile_pool(name="sqp", bufs=3) as sqp,
            tc.tile_pool(name="invp", bufs=3) as invp,
            tc.tile_pool(name="evp", bufs=2) as evp,
            tc.tile_pool(name="small", bufs=1) as smallp,
            tc.tile_pool(name="psb", bufs=4, space="PSUM") as psb,
            tc.tile_pool(name="psn", bufs=2, space="PSUM") as psn,
        ):
            ones128 = smallp.tile([128, 128], BF16)
            nc.vector.memset(ones128[:], 1.0)
            eps128 = smallp.tile([128, 1], F32)
            nc.vector.memset(eps128[:], 1e-20)

            # ---- inputs: fine pieces ordered for earliest compute start --
            f1 = pers.tile([C, HS, W], BF16)
            f2t = pers.tile([C, WP, HP], BF16)

            def f2_piece(i, eng):   # 8 pieces of 33 w-cols
                eng.dma_start(out=f2t[:, 33 * i:33 * i + 33, :],
                              in_=f2_d[:, 33 * i:33 * i + 33, :])

            def f1_piece(i, eng):   # 8 pieces of 8 h-rows
                eng.dma_start(out=f1[:, 8 * i:8 * i + 8, :],
                              in_=f1_d[:, 8 * i:8 * i + 8, :])

            # one FIFO ring drains in-order at full bandwidth: issue all
            # pieces on sync, critical-path data (f2 w<165, f1 h<8) first
            f2_piece(0, nc.sync)
            f1_piece(0, nc.sync)
            for i in range(1, 5):
                f2_piece(i, nc.sync)
            f1_piece(1, nc.sync)
            for i in range(5, 8):
                f2_piece(i, nc.sync)
            for i in range(2, 8):
                f1_piece(i, nc.sync)
            f1f = f1[:].rearrange("c h w -> c (h w)")
            f2f = f2t[:].rearrange("c w h -> c (w h)")

            band = pers.tile([128, 2 * HS * 360], BF16)

            # ---- normalization chunk pipeline ----
            # chunk list: (src_view, start, n, scale, sq_on_gpsimd)
            n2c = (N2 + CH - 1) // CH          # 19
            n1c = N1 // CH                     # 16
            chunks = []
            for i in range(n2c):
                n = min(CH, N2 - CH * i)
                # sq engine: gpsimd for off-critical rest chunks; ACT Square
                # for a few head chunks (DVE paces the head); else DVE
                sq_eng = 1 if (i >= 10 and i % 2 == 0) else \
                    (2 if i in (1, 3, 5) else 0)
                chunks.append(("f2", CH * i, n, 1.0, sq_eng, False))
            for i in range(n1c):
                chunks.append(("f1", CH * i, CH, float(C * C),
                               1 if i % 3 == 2 else 0, False))
            # emission order: f2[0:10] + f1[0] first (gram critical path),
            # rest interleaved into the gram loop.
            # rest: f1 chunks FIRST — gram (0, h) needs f1 chunk h//4
            # normalized by gram-group h (f1 chunk k lands at group k-1).
            head = chunks[:10] + [chunks[n2c]]
            rest = chunks[n2c + 1:] + chunks[10:n2c]

            pending = []

            def emit_norm(ck):
                kind, s0, n, scale, sq_eng, mul_gps = ck
                src = f2f if kind == "f2" else f1f
                sq = sqp.tile([128, CH], BF16)
                if sq_eng == 2:
                    nc.scalar.activation(
                        out=sq[:, 0:n], in_=src[:, s0:s0 + n],
                        func=mybir.ActivationFunctionType.Square)
                else:
                    eng = nc.gpsimd if sq_eng == 1 else nc.vector
                    eng.tensor_mul(out=sq[:, 0:n], in0=src[:, s0:s0 + n],
                                   in1=src[:, s0:s0 + n])
                ps = psn.tile([128, CH], F32)
                for off in range(0, n, 512):
                    m = min(512, n - off)
                    nc.tensor.matmul(ps[:, off:off + m], ones128[:],
                                     sq[:, off:off + m], start=True,
                                     stop=True)
                inv = invp.tile([128, CH], BF16)
                nc.scalar.activation(
                    out=inv[:, 0:n], in_=ps[:, 0:n],
                    func=mybir.ActivationFunctionType.Abs_reciprocal_sqrt,
                    scale=scale, bias=eps128[:])
                pending.append((src, s0, n, inv, mul_gps))

            def flush_mul():
                src, s0, n, inv, mul_gps = pending.pop(0)
                eng = nc.gpsimd if mul_gps else nc.vector
                eng.tensor_mul(out=src[:, s0:s0 + n],
                               in0=src[:, s0:s0 + n],
                               in1=inv[:, 0:n])

            for ck in head:
                emit_norm(ck)
                if len(pending) > 1:
                    flush_mul()
            while pending:      # f1 chunk 0 must be normalized pre-gram
                flush_mul()

            # ---- gram sweep + evac + chunked dump ----
            f2ap = f2t[:]

            def leaky_blk(b0, b1):
                blk = band[:, b0 * 360:b1 * 360]
                t = evp.tile([128, (b1 - b0) * 360], BF16)
                nc.vector.tensor_scalar_mul(out=t[:], in0=blk, scalar1=0.1)
                nc.vector.tensor_tensor(out=blk, in0=blk, in1=t[:],
                                        op=mybir.AluOpType.max)

            rest_i = 0
            for g in range(2):
                for h in range(HS):
                    # interleave remaining norm chunks every other group so
                    # the tensor FIFO isn't throttled by sum-matmuls
                    grp = HS * g + h
                    if rest_i < len(rest) and grp % 2 == 0:
                        emit_norm(rest[rest_i])
                        rest_i += 1
                    elif pending and grp % 2 == 1:
                        flush_mul()
                    pband = psb.tile([128, 360], F32)
                    for j in range(4):
                        w0 = 128 * g + 32 * j
                        rhs = bass.AP(
                            tensor=f2ap.tensor,
                            offset=f2ap.offset + w0 * HP + h,
                            ap=[[WP * HP, 128], [HP, 40], [1, 9]],
                        )
                        nc.tensor.matmul(
                            pband[32 * j:32 * j + 32, :],
                            f1[:, h, w0:w0 + 32],
                            rhs, start=True, stop=True,
                            tile_position=(0, 32 * j))
                    row = HS * g + h
                    r16 = h % 16
                    last_chunk = (g == 1 and h >= HS - 16)
                    if last_chunk:
                        # DVE rows first (+deferred leaky), ACT rows last,
                        # fine-grained dumps -> short tail
                        dst = band[:, row * 360:row * 360 + 360]
                        if r16 < 8:
                            nc.vector.tensor_scalar_mul(
                                out=dst, in0=pband[:], scalar1=1.0)
                        else:
                            nc.scalar.activation(
                                out=dst, in_=pband[:],
                                func=mybir.ActivationFunctionType.Prelu,
                                alpha=0.1)
                        if r16 == 7:
                            leaky_blk(112, 120)
                            nc.sync.dma_start(
                                out=bo_d[:, 112 * 360:120 * 360],
                                in_=band[:, 112 * 360:120 * 360])
                        elif r16 in (11, 15):
                            c0 = (120 if r16 == 11 else 124) * 360
                            c1 = (row + 1) * 360
                            nc.sync.dma_start(out=bo_d[:, c0:c1],
                                              in_=band[:, c0:c1])
                        continue
                    dst = band[:, row * 360:row * 360 + 360]
                    if r16 < ACT_ROWS:
                        nc.scalar.activation(
                            out=dst, in_=pband[:],
                            func=mybir.ActivationFunctionType.Prelu,
                            alpha=0.1)
                    else:
                        nc.vector.tensor_scalar_mul(
                            out=dst, in0=pband[:], scalar1=1.0)
                    if r16 == 15:
                        leaky_blk(row - (15 - ACT_ROWS), row + 1)
                        c0 = (row - 15) * 360
                        nc.sync.dma_start(
                            out=bo_d[:, c0:(row + 1) * 360],
                            in_=band[:, c0:(row + 1) * 360])
            while pending:
                flush_mul()

    nc.compile()
    return nc


def _get_nc():
    if "nc" not in _CACHE:
        _CACHE["nc"] = _build()
    return _CACHE["nc"]


def _shard_inputs(feat1, feat2_warped):
    feat1 = np.asarray(feat1, dtype=np.float32)
    feat2 = np.asarray(feat2_warped, dtype=np.float32)
    f2pad = np.pad(feat2, ((0, 0), (0, 0), (D, D), (D, D)))
    in_maps = []
    for k in range(8):
        b, s = k // 2, k % 2
        f1s = feat1[b, :, HS * s: HS * s + HS, :].astype(BF16_NP)
        f2s = f2pad[b, :, HS * s: HS * s + HP, :]          # [C, 72, 264]
        f2s = np.ascontiguousarray(
            f2s.transpose(0, 2, 1)).astype(BF16_NP)        # [C, 264, 72]
        in_maps.append({"f1": np.ascontiguousarray(f1s), "f2t": f2s})
    return in_maps


_GIDX = (9 * (np.arange(128) % 32))[:, None, None, None] + \
    np.arange(81)[None, None, None, :]  # [128,1,1,81]


def _gather(results):
    out = np.empty((B, 81, H, W), dtype=np.float32)
    for k in range(8):
        b, s = k // 2, k % 2
        band = results[k]["bandout"].astype(np.float32)  # [128, 2*HS*360]
        band = band.reshape(128, 2, HS, 360)             # p, g, h, (u' dy)
        # pixel w = 128g + p; value (dy,dx) at col (p%32 + dx)*9 + dy
        bw = np.take_along_axis(band, _GIDX, axis=3)     # [128,2,HS,81]
        bw = bw.reshape(128, 2, HS, 9, 9)                # p g h dx dy
        core = bw.transpose(4, 3, 2, 1, 0).reshape(81, HS, W)
        out[b, :, HS * s: HS * s + HS, :] = core
    return out


def run(feat1, feat2_warped, trace=False):
    nc = _get_nc()
    in_maps = _shard_inputs(feat1, feat2_warped)
    res = run_bass_kernel_spmd(nc, in_maps, list(range(8)), trace=trace)
    return _gather(res.results), res


def kernel(feat1, feat2_warped):
    out, _ = run(feat1, feat2_warped)
    return out
